# revision 20
# baseline (speedup 1.0000x reference)
"""Multi-head attention (B=2, S=2048, E=1024, H=16, D=64) on 8 TRN2 cores.

Sharding: core c handles batch b = c//4 and head-group g = c%4 (4 heads,
256 embed cols). No cross-core communication; host slices inputs (pre-
transposed and pre-cast to bf16) and gathers/normalizes outputs.

Per-core device program (fp16 matmuls, fp32 PSUM accumulation):
  - projections: qhT[c, s] = sum_e wq[e, c] qT[e, s] (c on partitions), so
    attention needs no on-chip transposes; K-bias dropped (softmax-invariant),
    V-bias applied on host (distributes through softmax).  et-outer loops so
    MMs trail the DMA stream; kT/qT load before vT so scores start early.
  - attention processes head PAIRS x iq PAIRS: scores matmuls run
    concurrently on disjoint PE row groups (K=64 each) into [128, 1024]
    PSUM tiles; the exp is split between ScalarE (table exp) and VectorE
    (Schraudolph int16 bit-trick, +-3% per element) to double softmax
    throughput; out matmuls reuse vh weights across the iq pair.
  - vh carries a ones column (m=65), so the out-stage accumulates the
    softmax denominator in PSUM row 64; host divides.
"""

import sys

sys.path.insert(0, "/opt/trn_rl_repo")

import os

import numpy as np

if os.environ.get("JAX_PLATFORMS") == "cpu":
    # the bass program must run on the neuron cores; the axon/neuron PJRT
    # platform registers only when JAX_PLATFORMS is unset/empty
    del os.environ["JAX_PLATFORMS"]

import concourse.bass as bass  # noqa: F401
import concourse.mybir as mybir
from concourse import bacc
from concourse.tile import TileContext

B, S, E = 2, 2048, 1024
H, D = 16, 64
HPC = 4  # heads per core
COLS = HPC * D  # 256
P = 128
F32 = mybir.dt.float32
F16 = mybir.dt.float16
ET = E // P  # 8 e-tiles
JT = S // P  # 16 j-tiles
NB = 512
NIQ = S // NB  # 4 i-quarters

_CACHED = {}


# Schraudolph fp16-bit exp for the DVE path: int16(s*EC1 + EC2) viewed as
# fp16 equals exp(0.125*s) within +-3%; the constant sits midway between the
# round- and truncate-cast optima so either convert semantic works.
EC1 = 0.125 * 1024.0 / float(np.log(2.0))  # 184.66494
EC2 = 15316.2
I16 = mybir.dt.int16


def build():
    nc = bacc.Bacc("TRN2", target_bir_lowering=False, debug=False)
    qT = nc.dram_tensor("qT", [E, S], F16, kind="ExternalInput")
    kT = nc.dram_tensor("kT", [E, S], F16, kind="ExternalInput")
    vT = nc.dram_tensor("vT", [E, S], F16, kind="ExternalInput")
    wq = nc.dram_tensor("wq", [E, COLS], F16, kind="ExternalInput")
    wk = nc.dram_tensor("wk", [E, COLS], F16, kind="ExternalInput")
    wv = nc.dram_tensor("wv", [E, COLS], F16, kind="ExternalInput")
    bq = nc.dram_tensor("bq", [P, 2], F32, kind="ExternalInput")
    # out_raw[:, (h*NIQ+iq)*NB : ...]: rows 0-63 numerator (d), row 64 denom
    out_raw = nc.dram_tensor("out_raw", [65, HPC * S], F32,
                             kind="ExternalOutput")  # [65, 8192]

    with TileContext(nc) as tc:
        with (
            tc.tile_pool(name="wp", bufs=1) as wp,
            tc.tile_pool(name="xq", bufs=ET) as xq,
            tc.tile_pool(name="xk", bufs=ET) as xk,
            tc.tile_pool(name="xv", bufs=ET) as xv,
            tc.tile_pool(name="hp", bufs=1) as hp,
            tc.tile_pool(name="pe", bufs=24) as pe,
            tc.tile_pool(name="ob", bufs=4) as ob,
            tc.tile_pool(name="psA", bufs=2, space="PSUM") as psA,
            tc.tile_pool(name="psO", bufs=4, space="PSUM") as psO,
        ):
            # --- DMA order: bias, wk+kT, wq+qT, wv+vT.  Projections are
            # et-outer so MMs trail each arriving x tile; q before v so
            # scores+exp warm up while vT streams.
            bq_t = wp.tile([P, 2], F32)
            nc.sync.dma_start(bq_t, bq[:, :])
            wq_b = wp.tile([P, ET, COLS], F16)
            wk_b = wp.tile([P, ET, COLS], F16)
            wv_b = wp.tile([P, ET, COLS], F16)

            def load_x(pool, tag):
                return [
                    pool.tile([P, S], F16, tag=tag, name=f"{tag}{et}")
                    for et in range(ET)
                ]

            qx, kx, vx = load_x(xq, "qx"), load_x(xk, "kx"), load_x(xv, "vx")
            # interleave wk chunks with kx tiles: K-proj et-step needs only
            # wk[:, et] + kx[et], so MMs start ~3us earlier than waiting for
            # the whole strided wk rearrange
            for et in range(ET):
                nc.sync.dma_start(wk_b[:, et, :], wk[et * P : (et + 1) * P, :])
                nc.sync.dma_start(kx[et], kT[et * P : (et + 1) * P, :])
            nc.sync.dma_start(wq_b, wq.rearrange("(t p) c -> p t c", p=P))
            for et in range(ET):
                nc.sync.dma_start(qx[et], qT[et * P : (et + 1) * P, :])
            nc.sync.dma_start(wv_b, wv.rearrange("(t p) c -> p t c", p=P))
            for et in range(ET):
                nc.sync.dma_start(vx[et], vT[et * P : (et + 1) * P, :])

            # --- resident head tensors ---
            qhT = hp.tile([P, 2, S], F16)  # [2 heads x 64 d, chunk, s]
            khT = hp.tile([P, 2, S], F16)
            vh_aug = hp.tile([P, JT, HPC * 65], F16)
            warm = pe.tile([P, 1024], F16, tag="e", name="warm")
            nc.scalar.activation(
                warm[:, 0:2], bq_t, mybir.ActivationFunctionType.Exp, scale=0.0
            )
            nc.vector.tensor_copy(vh_aug[:, 0, 0:2], warm[:, 0:2])
            nc.vector.memset(vh_aug, 1.0)

            # --- K/Q projections, et-outer: 4 sb accumulators live across
            # the et sweep so the first MMs only need x[0]; weights are
            # reused across the 4 slots per et step.
            def qk_proj(name, w_b, x, dst, bias, ch):
                pss = [
                    psO.tile([P, NB], F32, tag="o", name=f"ps_{name}{ch}{sb}")
                    for sb in range(4)
                ]
                for et in range(ET):
                    for sb in range(4):
                        nc.tensor.matmul(
                            pss[sb],
                            w_b[:, et, ch * P : (ch + 1) * P],
                            x[et][:, sb * NB : (sb + 1) * NB],
                            start=(et == 0),
                            stop=(et == ET - 1),
                        )
                for sb in range(4):
                    if bias is not None:
                        nc.vector.tensor_scalar_add(
                            dst[:, ch, sb * NB : (sb + 1) * NB],
                            pss[sb],
                            bias[:, ch : ch + 1],
                        )
                    else:
                        nc.vector.tensor_copy(
                            dst[:, ch, sb * NB : (sb + 1) * NB], pss[sb]
                        )

            qk_proj("k", wk_b, kx, khT, None, ch=0)
            qk_proj("k", wk_b, kx, khT, None, ch=1)
            qk_proj("q", wq_b, qx, qhT, bq_t, ch=0)
            qk_proj("q", wq_b, qx, qhT, bq_t, ch=1)

            # --- V projection, et-outer in 4 passes of 4 s-tiles.  Each
            # slot owns a full psum bank (cols 0-255 used): sharing a bank
            # between two accumulation groups corrupts the accumulation.
            for vp in range(4):
                psv = [
                    psO.tile([P, NB], F32, tag="o", name=f"ps_v{vp}{j}")
                    for j in range(4)
                ]
                for et in range(ET):
                    for j in range(4):
                        sc = vp * 4 + j
                        nc.tensor.matmul(
                            psv[j][:, :COLS],
                            vx[et][:, sc * P : (sc + 1) * P],
                            wv_b[:, et, :],
                            start=(et == 0),
                            stop=(et == ET - 1),
                        )
                for j in range(4):
                    sc = vp * 4 + j
                    nc.vector.tensor_copy(
                        vh_aug[:, sc].rearrange("p (h x) -> p h x", x=65)[:, :, :D],
                        psv[j][:, :COLS].rearrange("p (h x) -> p h x", x=D),
                    )

            # --- attention: blocks of (head-pair, iq-pair, jt).  Per block:
            # 2 concurrent-pair scores MMs per iq (khT weights reused across
            # both iq), one exp per iq split across ScalarE (table exp) and
            # VectorE (Schraudolph int16 bit-trick), and 4 out MMs (vh
            # weights reused across the iq pair).  Out lags scores by LAG
            # blocks so exp latency stays off the PE critical path; the lag
            # starts large (V projection still streaming) and drains to 2.
            from collections import deque

            blocks = [(pr, iqp, jt) for pr in range(2) for iqp in range(2)
                      for jt in range(JT)]
            ops = {}  # (pr, iqp) -> [op(h0,a), op(h0,b), op(h1,a), op(h1,b)]
            pending = deque()

            def emit_out_half(pr, iqp, jt, hh, ea, eb):
                """Two out MMs (one head, both iq) — emitted between scores
                pairs so the out streams hide scores LDW/drain stalls."""
                if jt == 0 and hh == 0:
                    # allocate here (not at scores emission): all previous
                    # sweeps' out MMs + evacs are already emitted, so the
                    # pool sees every reader of the recycled psum banks
                    ops[(pr, iqp)] = [
                        psO.tile([P, NB], F32, tag="o", name=f"op{k}")
                        for k in range(4)
                    ]
                opt = ops[(pr, iqp)]
                h = 2 * pr + hh
                for k, e in ((0, ea), (1, eb)):
                    nc.tensor.matmul(
                        opt[hh * 2 + k][:65, :],
                        vh_aug[:, jt, h * 65 : (h + 1) * 65],
                        e[:, hh * NB : (hh + 1) * NB],
                        start=(jt == 0),
                        stop=(jt == JT - 1),
                    )
                if jt == JT - 1:  # evacuate + store this half's outputs
                    for k in range(2):
                        iq = iqp * 2 + k
                        r = (2 * pr + hh) * NIQ + iq
                        osb = ob.tile([P, NB], F32, tag="ob", name="osb")
                        if (hh + k) % 2 == 0:
                            nc.scalar.copy(osb[:65, :], opt[hh * 2 + k][:65, :])
                        else:
                            nc.vector.tensor_copy(
                                osb[:65, :], opt[hh * 2 + k][:65, :]
                            )
                        nc.sync.dma_start(
                            out_raw[:, r * NB : (r + 1) * NB], osb[:65, :]
                        )
                    if hh == 1:
                        del ops[(pr, iqp)]

            for bi, (pr, iqp, jt) in enumerate(blocks):
                exps = []
                for k in range(2):
                    iq = iqp * 2 + k
                    sps = psA.tile([P, 1024], F32, tag="s", name=f"sps{k}")
                    for hh in range(2):  # row-group packed, concurrent
                        r0 = hh * D
                        nc.tensor.matmul(
                            sps[:, hh * NB : (hh + 1) * NB],
                            khT[r0 : r0 + D, pr, jt * P : (jt + 1) * P],
                            qhT[r0 : r0 + D, pr, iq * NB : (iq + 1) * NB],
                            start=True,
                            stop=True,
                        )
                    expT = pe.tile([P, 1024], F16, tag="e", name=f"expT{k}")
                    # 72 ACT / 56 DVE split balances the engines (ScalarE
                    # also does out evacuations); same max error as 50/50
                    if (bi + k) % 2 == 0 or bi % 8 == 7:
                        nc.scalar.activation(
                            expT, sps, mybir.ActivationFunctionType.Exp,
                            scale=0.125,
                        )
                    else:
                        nc.vector.tensor_scalar(
                            expT[:, :].bitcast(I16), sps, EC1, EC2,
                            op0=mybir.AluOpType.mult, op1=mybir.AluOpType.add,
                        )
                    exps.append(expT)
                    # out halves interleave between the two scores pairs so
                    # the out streams cover scores LDW + drain latency; the
                    # lag shrinks near the end so the tail drains tight
                    if bi < 23:
                        lag = max(4, 22 - 2 * max(0, bi - 11))
                    else:
                        lag = 4 if bi < 60 else 2
                    while len(pending) > lag:
                        emit_out_half(*pending.popleft())
                pending.append((pr, iqp, jt, 0, exps[0], exps[1]))
                pending.append((pr, iqp, jt, 1, exps[0], exps[1]))
            while pending:
                emit_out_half(*pending.popleft())
    nc.finalize()
    return nc


def _prep_in_maps(q, k, v, wq, bq, wk, bk, wv, bv):
    bf = np.float16
    q, k, v = (np.asarray(x, np.float32) for x in (q, k, v))
    wqb, wkb, wvb = (np.asarray(x, bf) for x in (wq, wk, wv))
    bq = np.asarray(bq, np.float32)
    qT = [np.ascontiguousarray(q[b].T.astype(bf)) for b in range(B)]
    kT = [np.ascontiguousarray(k[b].T.astype(bf)) for b in range(B)]
    vT = [np.ascontiguousarray(v[b].T.astype(bf)) for b in range(B)]
    in_maps = []
    for c in range(8):
        b, g = divmod(c, 4)
        cs = slice(g * COLS, (g + 1) * COLS)
        in_maps.append(
            {
                "qT": qT[b],
                "kT": kT[b],
                "vT": vT[b],
                "wq": np.ascontiguousarray(wqb[:, cs]),
                "wk": np.ascontiguousarray(wkb[:, cs]),
                "wv": np.ascontiguousarray(wvb[:, cs]),
                "bq": np.ascontiguousarray(bq[cs].reshape(2, P).T),
            }
        )
    return in_maps


def _make_runner(nc, n_cores=8):
    """Persistent jitted shard_map runner over the prebuilt Bass module."""
    import jax
    from jax.experimental.shard_map import shard_map
    from jax.sharding import Mesh, NamedSharding, PartitionSpec
    from concourse import bass2jax

    bass2jax.install_neuronx_cc_hook()

    in_names, out_names, out_avals, zero_outs = [], [], [], []
    for alloc in nc.m.functions[0].allocations:
        if not isinstance(alloc, mybir.MemoryLocationSet):
            continue
        name = alloc.memorylocations[0].name
        if alloc.kind == "ExternalInput":
            in_names.append(name)
        elif alloc.kind == "ExternalOutput":
            shape = tuple(alloc.tensor_shape)
            dtype = mybir.dt.np(alloc.dtype)
            out_avals.append(jax.core.ShapedArray(shape, dtype))
            zero_outs.append(np.zeros((n_cores * shape[0], *shape[1:]), dtype))
            out_names.append(name)
    pid_name = nc.partition_id_tensor.name if nc.partition_id_tensor else None
    if pid_name is not None:
        in_names = [n for n in in_names if n != pid_name]
    n_params = len(in_names)
    all_names = in_names + out_names + ([pid_name] if pid_name else [])

    def _body(*args):
        operands = list(args)
        if pid_name is not None:
            operands.append(bass2jax.partition_id_tensor())
        outs = bass2jax._bass_exec_p.bind(
            *operands,
            out_avals=tuple(out_avals),
            in_names=tuple(all_names),
            out_names=tuple(out_names),
            lowering_input_output_aliases=(),
            sim_require_finite=True,
            sim_require_nnan=True,
            nc=nc,
        )
        return tuple(outs)

    devices = jax.devices()[:n_cores]
    mesh = Mesh(np.asarray(devices), ("core",))
    nio = n_params + len(out_names)
    sharded = jax.jit(
        shard_map(
            _body,
            mesh=mesh,
            in_specs=(PartitionSpec("core"),) * nio,
            out_specs=(PartitionSpec("core"),) * len(out_names),
            check_rep=False,
        ),
        keep_unused=True,
    )
    row_sharding = NamedSharding(mesh, PartitionSpec("core"))
    zeros_dev = [jax.device_put(z, row_sharding) for z in zero_outs]

    def run(in_maps):
        concat_in = [
            np.concatenate([np.asarray(m[name]) for m in in_maps], axis=0)
            for name in in_names
        ]
        out_arrs = sharded(*concat_in, *zeros_dev)
        return [
            {
                name: np.asarray(out_arrs[i]).reshape(n_cores, *out_avals[i].shape)[c]
                for i, name in enumerate(out_names)
            }
            for c in range(n_cores)
        ]

    run.sharded = sharded
    run.in_names = in_names
    run.zeros_dev = zeros_dev
    run.row_sharding = row_sharding
    return run


def get_runner():
    if "run" not in _CACHED:
        _CACHED["nc"] = build()
        _CACHED["run"] = _make_runner(_CACHED["nc"])
    return _CACHED["run"]


def kernel(q, k, v, wq, bq, wk, bk, wv, bv):
    run = get_runner()
    in_maps = _prep_in_maps(q, k, v, wq, bq, wk, bk, wv, bv)
    results = run(in_maps)

    bv = np.asarray(bv, np.float32)
    out = np.empty((B, S, E), np.float32)
    for c in range(8):
        b, g = divmod(c, 4)
        raw = results[c]["out_raw"]  # [65, 8192]
        num = raw[:64].reshape(64, HPC, S)  # [d, h, i] (NIQ*NB = S)
        den = raw[64].reshape(HPC, S)
        for h in range(HPC):
            col0 = g * COLS + h * D
            o = num[:, h, :] / den[h][None, :]
            out[b, :, col0 : col0 + D] = o.T + bv[col0 : col0 + D][None, :]
    return out



# revision 21
# speedup vs baseline: 1.1930x; 1.1930x over previous
"""Multi-head attention (B=2, S=2048, E=1024, H=16, D=64) on 8 TRN2 cores.

Sharding: core c handles batch b = c//4 and head-group g = c%4 (4 heads,
256 embed cols). No cross-core communication; host slices inputs (pre-
transposed and pre-cast to bf16) and gathers/normalizes outputs.

Per-core device program (fp16 matmuls, fp32 PSUM accumulation):
  - projections: qhT[c, s] = sum_e wq[e, c] qT[e, s] (c on partitions), so
    attention needs no on-chip transposes; K-bias dropped (softmax-invariant),
    V-bias applied on host (distributes through softmax).  et-outer loops so
    MMs trail the DMA stream; kT/qT load before vT so scores start early.
  - attention processes head PAIRS x iq PAIRS: scores matmuls run
    concurrently on disjoint PE row groups (K=64 each) into [128, 1024]
    PSUM tiles; the exp is split between ScalarE (table exp) and VectorE
    (Schraudolph int16 bit-trick, +-3% per element) to double softmax
    throughput; out matmuls reuse vh weights across the iq pair.
  - vh carries a ones column (m=65), so the out-stage accumulates the
    softmax denominator in PSUM row 64; host divides.
"""

import sys

sys.path.insert(0, "/opt/trn_rl_repo")

import os

import numpy as np

if os.environ.get("JAX_PLATFORMS") == "cpu":
    # the bass program must run on the neuron cores; the axon/neuron PJRT
    # platform registers only when JAX_PLATFORMS is unset/empty
    del os.environ["JAX_PLATFORMS"]

import concourse.bass as bass  # noqa: F401
import concourse.mybir as mybir
from concourse import bacc
from concourse.tile import TileContext

B, S, E = 2, 2048, 1024
H, D = 16, 64
HPC = 4  # heads per core
COLS = HPC * D  # 256
P = 128
F32 = mybir.dt.float32
F16 = mybir.dt.float16
ET = E // P  # 8 e-tiles
JT = S // P  # 16 j-tiles
NB = 512
NIQ = S // NB  # 4 i-quarters

_CACHED = {}


# Schraudolph fp16-bit exp for the DVE path: int16(s*EC1 + EC2) viewed as
# fp16 equals exp(0.125*s) within +-3%; the constant sits midway between the
# round- and truncate-cast optima so either convert semantic works.
EC1 = 0.125 * 1024.0 / float(np.log(2.0))  # 184.66494
EC2 = 15316.2
I16 = mybir.dt.int16


def build():
    nc = bacc.Bacc("TRN2", target_bir_lowering=False, debug=False)
    qT = nc.dram_tensor("qT", [E, S], F16, kind="ExternalInput")
    kT = nc.dram_tensor("kT", [E, S], F16, kind="ExternalInput")
    vT = nc.dram_tensor("vT", [E, S], F16, kind="ExternalInput")
    wq = nc.dram_tensor("wq", [E, COLS], F16, kind="ExternalInput")
    wk = nc.dram_tensor("wk", [E, COLS], F16, kind="ExternalInput")
    wv = nc.dram_tensor("wv", [E, COLS], F16, kind="ExternalInput")
    bq = nc.dram_tensor("bq", [P, 2], F32, kind="ExternalInput")
    # out_raw[:, (h*NIQ+iq)*NB : ...]: rows 0-63 numerator (d), row 64 denom
    out_raw = nc.dram_tensor("out_raw", [65, HPC * S], F32,
                             kind="ExternalOutput")  # [65, 8192]

    with TileContext(nc) as tc:
        with (
            tc.tile_pool(name="wp", bufs=1) as wp,
            tc.tile_pool(name="xq", bufs=ET) as xq,
            tc.tile_pool(name="xk", bufs=ET) as xk,
            tc.tile_pool(name="xv", bufs=ET) as xv,
            tc.tile_pool(name="hp", bufs=1) as hp,
            tc.tile_pool(name="pe", bufs=24) as pe,
            tc.tile_pool(name="ob", bufs=4) as ob,
            tc.tile_pool(name="psA", bufs=2, space="PSUM") as psA,
            tc.tile_pool(name="psO", bufs=4, space="PSUM") as psO,
        ):
            # --- DMA order: bias, wk+kT, wq+qT, wv+vT.  Projections are
            # et-outer so MMs trail each arriving x tile; q before v so
            # scores+exp warm up while vT streams.
            bq_t = wp.tile([P, 2], F32)
            nc.sync.dma_start(bq_t, bq[:, :])
            wq_b = wp.tile([P, ET, COLS], F16)
            wk_b = wp.tile([P, ET, COLS], F16)
            wv_b = wp.tile([P, ET, COLS], F16)

            def load_x(pool, tag):
                return [
                    pool.tile([P, S], F16, tag=tag, name=f"{tag}{et}")
                    for et in range(ET)
                ]

            qx, kx, vx = load_x(xq, "qx"), load_x(xk, "kx"), load_x(xv, "vx")
            # interleave wk chunks with kx tiles: K-proj et-step needs only
            # wk[:, et] + kx[et], so MMs start ~3us earlier than waiting for
            # the whole strided wk rearrange
            for et in range(ET):
                nc.sync.dma_start(wk_b[:, et, :], wk[et * P : (et + 1) * P, :])
                nc.sync.dma_start(kx[et], kT[et * P : (et + 1) * P, :])
            nc.sync.dma_start(wq_b, wq.rearrange("(t p) c -> p t c", p=P))
            for et in range(ET):
                nc.sync.dma_start(qx[et], qT[et * P : (et + 1) * P, :])
            nc.sync.dma_start(wv_b, wv.rearrange("(t p) c -> p t c", p=P))
            for et in range(ET):
                nc.sync.dma_start(vx[et], vT[et * P : (et + 1) * P, :])

            # --- resident head tensors ---
            qhT = hp.tile([P, 2, S], F16)  # [2 heads x 64 d, chunk, s]
            khT = hp.tile([P, 2, S], F16)
            vh_aug = hp.tile([P, JT, HPC * 65], F16)
            warm = pe.tile([P, 1024], F16, tag="e", name="warm")
            nc.scalar.activation(
                warm[:, 0:2], bq_t, mybir.ActivationFunctionType.Exp, scale=0.0
            )
            nc.vector.tensor_copy(vh_aug[:, 0, 0:2], warm[:, 0:2])
            nc.vector.memset(vh_aug, 1.0)

            # --- K/Q projections, et-outer: 4 sb accumulators live across
            # the et sweep so the first MMs only need x[0]; weights are
            # reused across the 4 slots per et step.
            def qk_proj(name, w_b, x, dst, bias, ch):
                pss = [
                    psO.tile([P, NB], F32, tag="o", name=f"ps_{name}{ch}{sb}")
                    for sb in range(4)
                ]
                for et in range(ET):
                    for sb in range(4):
                        nc.tensor.matmul(
                            pss[sb],
                            w_b[:, et, ch * P : (ch + 1) * P],
                            x[et][:, sb * NB : (sb + 1) * NB],
                            start=(et == 0),
                            stop=(et == ET - 1),
                        )
                for sb in range(4):
                    if bias is not None:
                        nc.vector.tensor_scalar_add(
                            dst[:, ch, sb * NB : (sb + 1) * NB],
                            pss[sb],
                            bias[:, ch : ch + 1],
                        )
                    else:
                        nc.vector.tensor_copy(
                            dst[:, ch, sb * NB : (sb + 1) * NB], pss[sb]
                        )

            qk_proj("k", wk_b, kx, khT, None, ch=0)
            qk_proj("k", wk_b, kx, khT, None, ch=1)
            qk_proj("q", wq_b, qx, qhT, bq_t, ch=0)
            qk_proj("q", wq_b, qx, qhT, bq_t, ch=1)

            # --- V projection, et-outer in 4 passes of 4 s-tiles.  Each
            # slot owns a full psum bank (cols 0-255 used): sharing a bank
            # between two accumulation groups corrupts the accumulation.
            for vp in range(4):
                psv = [
                    psO.tile([P, NB], F32, tag="o", name=f"ps_v{vp}{j}")
                    for j in range(4)
                ]
                for et in range(ET):
                    for j in range(4):
                        sc = vp * 4 + j
                        nc.tensor.matmul(
                            psv[j][:, :COLS],
                            vx[et][:, sc * P : (sc + 1) * P],
                            wv_b[:, et, :],
                            start=(et == 0),
                            stop=(et == ET - 1),
                        )
                for j in range(4):
                    sc = vp * 4 + j
                    nc.vector.tensor_copy(
                        vh_aug[:, sc].rearrange("p (h x) -> p h x", x=65)[:, :, :D],
                        psv[j][:, :COLS].rearrange("p (h x) -> p h x", x=D),
                    )

            # --- attention: blocks of (head-pair, iq-pair, jt).  Per block:
            # 2 concurrent-pair scores MMs per iq (khT weights reused across
            # both iq), one exp per iq split across ScalarE (table exp) and
            # VectorE (Schraudolph int16 bit-trick), and 4 out MMs (vh
            # weights reused across the iq pair).  Out lags scores by LAG
            # blocks so exp latency stays off the PE critical path; the lag
            # starts large (V projection still streaming) and drains to 2.
            from collections import deque

            blocks = [(pr, iqp, jt) for pr in range(2) for iqp in range(2)
                      for jt in range(JT)]
            ops = {}  # (pr, iqp) -> [op(h0,a), op(h0,b), op(h1,a), op(h1,b)]
            pending = deque()

            def emit_out_half(pr, iqp, jt, hh, ea, eb):
                """Two out MMs (one head, both iq) — emitted between scores
                pairs so the out streams hide scores LDW/drain stalls."""
                if jt == 0 and hh == 0:
                    # allocate here (not at scores emission): all previous
                    # sweeps' out MMs + evacs are already emitted, so the
                    # pool sees every reader of the recycled psum banks
                    ops[(pr, iqp)] = [
                        psO.tile([P, NB], F32, tag="o", name=f"op{k}")
                        for k in range(4)
                    ]
                opt = ops[(pr, iqp)]
                h = 2 * pr + hh
                for k, e in ((0, ea), (1, eb)):
                    nc.tensor.matmul(
                        opt[hh * 2 + k][:65, :],
                        vh_aug[:, jt, h * 65 : (h + 1) * 65],
                        e[:, hh * NB : (hh + 1) * NB],
                        start=(jt == 0),
                        stop=(jt == JT - 1),
                    )
                if jt == JT - 1:  # evacuate + store this half's outputs
                    for k in range(2):
                        iq = iqp * 2 + k
                        r = (2 * pr + hh) * NIQ + iq
                        osb = ob.tile([P, NB], F32, tag="ob", name="osb")
                        if (hh + k) % 2 == 0:
                            nc.scalar.copy(osb[:65, :], opt[hh * 2 + k][:65, :])
                        else:
                            nc.vector.tensor_copy(
                                osb[:65, :], opt[hh * 2 + k][:65, :]
                            )
                        nc.sync.dma_start(
                            out_raw[:, r * NB : (r + 1) * NB], osb[:65, :]
                        )
                    if hh == 1:
                        del ops[(pr, iqp)]

            for bi, (pr, iqp, jt) in enumerate(blocks):
                exps = []
                for k in range(2):
                    iq = iqp * 2 + k
                    sps = psA.tile([P, 1024], F32, tag="s", name=f"sps{k}")
                    for hh in range(2):  # row-group packed, concurrent
                        r0 = hh * D
                        nc.tensor.matmul(
                            sps[:, hh * NB : (hh + 1) * NB],
                            khT[r0 : r0 + D, pr, jt * P : (jt + 1) * P],
                            qhT[r0 : r0 + D, pr, iq * NB : (iq + 1) * NB],
                            start=True,
                            stop=True,
                        )
                    expT = pe.tile([P, 1024], F16, tag="e", name=f"expT{k}")
                    if (bi + k) % 2 == 0:
                        nc.scalar.activation(
                            expT, sps, mybir.ActivationFunctionType.Exp,
                            scale=0.125,
                        )
                    else:
                        nc.vector.tensor_scalar(
                            expT[:, :].bitcast(I16), sps, EC1, EC2,
                            op0=mybir.AluOpType.mult, op1=mybir.AluOpType.add,
                        )
                    exps.append(expT)
                    # out halves interleave between the two scores pairs so
                    # the out streams cover scores LDW + drain latency; the
                    # lag shrinks near the end so the tail drains tight
                    if bi < 23:
                        lag = max(4, 22 - 2 * max(0, bi - 11))
                    else:
                        lag = 4 if bi < 60 else 2
                    while len(pending) > lag:
                        emit_out_half(*pending.popleft())
                pending.append((pr, iqp, jt, 0, exps[0], exps[1]))
                pending.append((pr, iqp, jt, 1, exps[0], exps[1]))
            while pending:
                emit_out_half(*pending.popleft())
    nc.finalize()
    return nc


def _prep_in_maps(q, k, v, wq, bq, wk, bk, wv, bv):
    bf = np.float16
    q, k, v = (np.asarray(x, np.float32) for x in (q, k, v))
    wqb, wkb, wvb = (np.asarray(x, bf) for x in (wq, wk, wv))
    bq = np.asarray(bq, np.float32)
    qT = [np.ascontiguousarray(q[b].T.astype(bf)) for b in range(B)]
    kT = [np.ascontiguousarray(k[b].T.astype(bf)) for b in range(B)]
    vT = [np.ascontiguousarray(v[b].T.astype(bf)) for b in range(B)]
    in_maps = []
    for c in range(8):
        b, g = divmod(c, 4)
        cs = slice(g * COLS, (g + 1) * COLS)
        in_maps.append(
            {
                "qT": qT[b],
                "kT": kT[b],
                "vT": vT[b],
                "wq": np.ascontiguousarray(wqb[:, cs]),
                "wk": np.ascontiguousarray(wkb[:, cs]),
                "wv": np.ascontiguousarray(wvb[:, cs]),
                "bq": np.ascontiguousarray(bq[cs].reshape(2, P).T),
            }
        )
    return in_maps


def _make_runner(nc, n_cores=8):
    """Persistent jitted shard_map runner over the prebuilt Bass module."""
    import jax
    from jax.experimental.shard_map import shard_map
    from jax.sharding import Mesh, NamedSharding, PartitionSpec
    from concourse import bass2jax

    bass2jax.install_neuronx_cc_hook()

    in_names, out_names, out_avals, zero_outs = [], [], [], []
    for alloc in nc.m.functions[0].allocations:
        if not isinstance(alloc, mybir.MemoryLocationSet):
            continue
        name = alloc.memorylocations[0].name
        if alloc.kind == "ExternalInput":
            in_names.append(name)
        elif alloc.kind == "ExternalOutput":
            shape = tuple(alloc.tensor_shape)
            dtype = mybir.dt.np(alloc.dtype)
            out_avals.append(jax.core.ShapedArray(shape, dtype))
            zero_outs.append(np.zeros((n_cores * shape[0], *shape[1:]), dtype))
            out_names.append(name)
    pid_name = nc.partition_id_tensor.name if nc.partition_id_tensor else None
    if pid_name is not None:
        in_names = [n for n in in_names if n != pid_name]
    n_params = len(in_names)
    all_names = in_names + out_names + ([pid_name] if pid_name else [])

    def _body(*args):
        operands = list(args)
        if pid_name is not None:
            operands.append(bass2jax.partition_id_tensor())
        outs = bass2jax._bass_exec_p.bind(
            *operands,
            out_avals=tuple(out_avals),
            in_names=tuple(all_names),
            out_names=tuple(out_names),
            lowering_input_output_aliases=(),
            sim_require_finite=True,
            sim_require_nnan=True,
            nc=nc,
        )
        return tuple(outs)

    devices = jax.devices()[:n_cores]
    mesh = Mesh(np.asarray(devices), ("core",))
    nio = n_params + len(out_names)
    sharded = jax.jit(
        shard_map(
            _body,
            mesh=mesh,
            in_specs=(PartitionSpec("core"),) * nio,
            out_specs=(PartitionSpec("core"),) * len(out_names),
            check_rep=False,
        ),
        keep_unused=True,
    )
    row_sharding = NamedSharding(mesh, PartitionSpec("core"))
    zeros_dev = [jax.device_put(z, row_sharding) for z in zero_outs]

    def run(in_maps):
        concat_in = [
            np.concatenate([np.asarray(m[name]) for m in in_maps], axis=0)
            for name in in_names
        ]
        out_arrs = sharded(*concat_in, *zeros_dev)
        return [
            {
                name: np.asarray(out_arrs[i]).reshape(n_cores, *out_avals[i].shape)[c]
                for i, name in enumerate(out_names)
            }
            for c in range(n_cores)
        ]

    run.sharded = sharded
    run.in_names = in_names
    run.zeros_dev = zeros_dev
    run.row_sharding = row_sharding
    return run


def get_runner():
    if "run" not in _CACHED:
        _CACHED["nc"] = build()
        _CACHED["run"] = _make_runner(_CACHED["nc"])
    return _CACHED["run"]


def kernel(q, k, v, wq, bq, wk, bk, wv, bv):
    run = get_runner()
    in_maps = _prep_in_maps(q, k, v, wq, bq, wk, bk, wv, bv)
    results = run(in_maps)

    bv = np.asarray(bv, np.float32)
    out = np.empty((B, S, E), np.float32)
    for c in range(8):
        b, g = divmod(c, 4)
        raw = results[c]["out_raw"]  # [65, 8192]
        num = raw[:64].reshape(64, HPC, S)  # [d, h, i] (NIQ*NB = S)
        den = raw[64].reshape(HPC, S)
        for h in range(HPC):
            col0 = g * COLS + h * D
            o = num[:, h, :] / den[h][None, :]
            out[b, :, col0 : col0 + D] = o.T + bv[col0 : col0 + D][None, :]
    return out

